# revision 29
# baseline (speedup 1.0000x reference)
"""CrossLevelAttention Trainium2 kernel (8 NeuronCores, Bass/Tile).

Strategy (cluster-sharded):
  * Host packs cells by cluster into uniform "lane" grids: each group is
    [128 lanes x S slots], a lane holds cells of exactly one cluster
    (clusters spanning multiple adjacent lanes are re-combined on device
    with a per-group 0/1 combine matrix through the PE).
  * Device streams the grids: k/v projections on the PE (features on
    partitions), per-head score reduction via a block-diagonal matmul,
    exp on the scalar engine, and the per-cluster segment sums
    (numerator and denominator) accumulate in PSUM across the slot
    stream.  Biases are folded out host-side (softmax is shift/scale
    invariant per cluster), so padded slots contribute exactly
    exp(0)=1 to the denominator, which is subtracted via a host-computed
    fill count.
  * Each core owns a disjoint set of clusters => no cross-core reduction
    for the segment sums; a single small AllGather shares the per-cluster
    top-down table.  The output pass is data-parallel over cells:
    dma_gather rows of the table by cluster label, dense stores.
"""

import os
import sys
import time
import numpy as np


def _log(msg):
    print(f"[kernel {time.strftime('%H:%M:%S')}] {msg}", file=sys.stderr, flush=True)

for _p in ("/opt/trn_rl_repo", "/root/.axon_site/_ro/trn_rl_repo"):
    if os.path.isdir(_p) and _p not in sys.path:
        sys.path.insert(0, _p)

from concourse import bass, bacc, mybir
import concourse.tile as tile
from concourse.bass_utils import run_bass_kernel_spmd

F32 = mybir.dt.float32
BF16 = mybir.dt.bfloat16
I16 = mybir.dt.int16

NCORES = 8
D = 128          # hidden
H = 8            # heads
DH = D // H      # head dim
S = 64           # slots per lane
LANES = 128      # lanes per group
B = 4            # slots per device batch (batch width = B*LANES = 512)
NB = S // B      # batches per group


# --------------------------------------------------------------------------
# host-side packing
# --------------------------------------------------------------------------

def _pack(labels, counts, order, starts, C):
    """Assign clusters to (group, lanes). Returns groups: list of list of
    (cluster, n_lanes), padded so len(groups) % NCORES == 0."""
    groups = []
    cur, cur_lanes = [], 0
    for c in range(C):
        cnt = int(counts[c])
        if cnt == 0:
            continue
        nl = -(-cnt // S)
        assert nl <= LANES, f"cluster {c} too large ({cnt} cells)"
        if cur_lanes + nl > LANES:
            groups.append(cur)
            cur, cur_lanes = [], 0
        cur.append((c, nl))
        cur_lanes += nl
    if cur:
        groups.append(cur)
    while len(groups) % NCORES != 0:
        groups.append([])
    return groups


def _host_prepare(cell_features, tissue_features, labels,
                  bu_in_w, bu_in_b, bu_out_w, bu_out_b,
                  td_in_w, td_in_b, td_out_w, td_out_b):
    N = cell_features.shape[0]
    C = tissue_features.shape[0]
    X = np.ascontiguousarray(cell_features, dtype=np.float32)
    tis = np.asarray(tissue_features, dtype=np.float32)
    labels = np.asarray(labels).astype(np.int64)
    valid = labels >= 0
    lab_v = np.where(valid, labels, 0)

    Wq, Wk, Wv = (np.asarray(bu_in_w[i * D:(i + 1) * D], np.float32) for i in range(3))
    bq, bk, bv = (np.asarray(bu_in_b[i * D:(i + 1) * D], np.float32) for i in range(3))
    Wv2 = np.asarray(td_in_w[2 * D:3 * D], np.float32)
    bv2 = np.asarray(td_in_b[2 * D:3 * D], np.float32)
    Wo = np.asarray(bu_out_w, np.float32)
    bo = np.asarray(bu_out_b, np.float32)
    Td = np.asarray(td_out_w, np.float32)
    tb = np.asarray(td_out_b, np.float32)

    scale = np.float32(1.0 / np.sqrt(np.float32(DH)))
    q = ((tis @ Wq.T) + bq) * scale                  # [C, D]
    bo_f = bv @ Wo.T + bo                            # folded v-bias -> out bias
    M2 = (Td @ Wv2).T                                # t_out = tn @ M2 + c2
    c2 = bv2 @ Td.T + tb

    # counts / CSR over valid cells only
    counts = np.bincount(labels[valid], minlength=C).astype(np.int64)
    order_all = np.argsort(np.where(valid, labels, C), kind="stable")
    order = order_all[: int(valid.sum())]
    starts = np.zeros(C + 1, np.int64)
    starts[1:] = np.cumsum(counts)

    groups = _pack(labels, counts, order, starts, C)
    G_total = len(groups)
    G_pc = G_total // NCORES

    Xp = np.vstack([X, np.zeros((1, D), np.float32)])   # filler row N -> zeros

    grids = np.zeros((G_total, D, S, LANES), np.float32)
    qT4s = np.zeros((G_total, D, B * LANES), np.float32)
    CMs = np.zeros((G_total, LANES, LANES), np.float32)
    nfills = np.zeros((G_total, H, LANES), np.float32)
    m01s = np.zeros((G_total, LANES, 1), np.float32)
    Hs = np.zeros((G_total, LANES, LANES), np.float32)
    tpos = np.zeros(C, np.int64)          # cluster -> row in gathered table
    slot_cluster = np.full((G_total, LANES), -1, np.int64)

    for g, members in enumerate(groups):
        ids = np.full((LANES, S), N, np.int64)
        qT = np.zeros((D, LANES), np.float32)
        lane0 = 0
        for o, (c, nl) in enumerate(members):
            cnt = int(counts[c])
            cells = order[starts[c]:starts[c] + cnt]
            buf = np.full(nl * S, N, np.int64)
            buf[:cnt] = cells
            ids[lane0:lane0 + nl] = buf.reshape(nl, S)
            CMs[g, lane0:lane0 + nl, o] = 1.0
            qT[:, lane0:lane0 + nl] = q[c][:, None]
            nfills[g, :, o] = nl * S - cnt
            m01s[g, o, 0] = 1.0
            Hs[g, o, :] = bo_f
            tpos[c] = g * LANES + o
            slot_cluster[g, o] = c
            lane0 += nl
        # grid: [lane, slot, feat] -> [feat, slot, lane]
        grids[g] = Xp[ids].transpose(2, 1, 0)
        qT4s[g] = np.tile(qT, (1, B))

    # ---- pass-4 index arrays (per core), int16 table positions ----
    TABLE = G_total * LANES
    assert TABLE < 32768
    per_core = -(-N // NCORES)            # cells per core (last core padded)
    per_core_pad = -(-per_core // 128) * 128
    calls = []                            # (row_start, nrows) within core slice
    # >2048 rows per dma_gather fails on HW (SWDGE ring); 1024 is safe.
    call_rows = int(os.environ.get("KGATHER", "1024"))
    a = 0
    while a < per_core_pad:
        nrows = min(call_rows, per_core_pad - a)
        calls.append((a, nrows))
        a += nrows
    gpos_all = tpos[lab_v]                # invalid labels -> cluster 0 (fixed later)

    idx_cols = per_core_pad // 16
    idxs = np.zeros((NCORES, 128, idx_cols), np.int16)
    for r in range(NCORES):
        lo = r * per_core
        sl = gpos_all[lo: lo + per_core]
        sl = np.concatenate([sl, np.full(per_core_pad - len(sl), sl[-1] if len(sl) else 0, np.int64)])
        col = 0
        for (a, nrows) in calls:  # noqa: B007 (layout matches device stores)
            gc = nrows // 128
            kidx = sl[a:a + nrows].reshape(128, gc)          # [p, g]
            lin = kidx.T.reshape(-1)                          # lin[g*128+p]
            wrapped = lin.reshape(-1, 16).T                   # [16, nrows/16]
            blk = np.tile(wrapped, (8, 1)).astype(np.int16)   # replicate to 128
            idxs[r, :, col: col + nrows // 16] = blk
            col += nrows // 16

    host = dict(
        N=N, C=C, G_total=G_total, G_pc=G_pc, per_core=per_core,
        per_core_pad=per_core_pad, calls=calls, idx_cols=idx_cols,
        groups=groups, slot_cluster=slot_cluster, counts=counts,
        valid=valid, tis=tis, X=X,
    )

    Sp = np.kron(np.eye(H, dtype=np.float32), np.ones((DH, DH), np.float32))
    B8 = np.kron(np.eye(H, dtype=np.float32), np.ones((DH, 1), np.float32)) / DH
    BT = np.kron(np.eye(H, dtype=np.float32), np.ones((1, DH), np.float32))

    if _prec() == "bf16":
        from ml_dtypes import bfloat16
        st = lambda a: np.ascontiguousarray(a.astype(bfloat16))
    else:
        st = np.ascontiguousarray

    shared = dict(
        wkt=st(Wk.T.copy()), wvt=st(Wv.T.copy()),
        sprime=st(Sp), b8=st(B8), ident=st(np.eye(128, dtype=np.float32)),
        bt=np.ascontiguousarray(BT),
        wot=np.ascontiguousarray(Wo.T), m2lhs=np.ascontiguousarray(M2),
        c2=np.ascontiguousarray(c2.reshape(D, 1)),
        i128=np.eye(128, dtype=np.float32),
    )

    per_core_maps = []
    for r in range(NCORES):
        s = slice(r * G_pc, (r + 1) * G_pc)
        m = dict(shared)
        m.update(
            grid=st(grids[s]),
            qt4=np.ascontiguousarray(qT4s[s]),
            cm=np.ascontiguousarray(CMs[s]),
            nfill=np.ascontiguousarray(nfills[s]),
            m01=np.ascontiguousarray(m01s[s]),
            hmat=np.ascontiguousarray(Hs[s]),
            idx=np.ascontiguousarray(idxs[r]),
        )
        per_core_maps.append(m)
    return host, per_core_maps


# --------------------------------------------------------------------------
# device program
# --------------------------------------------------------------------------

_CACHE = {}


def _build(G_pc, G_total, per_core_pad, idx_cols, calls, single=False):
    dbg = set(os.environ.get("KDBG", "").split(",")) - {""}
    key = (G_pc, G_total, per_core_pad, idx_cols, tuple(calls), single,
           tuple(sorted(dbg)))
    if key in _CACHE:
        return _CACHE[key]

    t0 = time.time()
    nc = bacc.Bacc("TRN2", target_bir_lowering=False, debug=False,
                   num_devices=1 if single else NCORES)

    BW = B * LANES  # 512

    grid = nc.dram_tensor("grid", [G_pc, D, S, LANES], F32, kind="ExternalInput").ap()
    qt4 = nc.dram_tensor("qt4", [G_pc, D, BW], F32, kind="ExternalInput").ap()
    cm = nc.dram_tensor("cm", [G_pc, LANES, LANES], F32, kind="ExternalInput").ap()
    nfill = nc.dram_tensor("nfill", [G_pc, H, LANES], F32, kind="ExternalInput").ap()
    m01 = nc.dram_tensor("m01", [G_pc, LANES, 1], F32, kind="ExternalInput").ap()
    hmat = nc.dram_tensor("hmat", [G_pc, LANES, LANES], F32, kind="ExternalInput").ap()
    idx = nc.dram_tensor("idx", [128, idx_cols], I16, kind="ExternalInput").ap()
    wkt = nc.dram_tensor("wkt", [D, D], F32, kind="ExternalInput").ap()
    wvt = nc.dram_tensor("wvt", [D, D], F32, kind="ExternalInput").ap()
    sprime = nc.dram_tensor("sprime", [D, D], F32, kind="ExternalInput").ap()
    b8 = nc.dram_tensor("b8", [D, H], F32, kind="ExternalInput").ap()
    bt = nc.dram_tensor("bt", [H, D], F32, kind="ExternalInput").ap()
    wot = nc.dram_tensor("wot", [D, D], F32, kind="ExternalInput").ap()
    m2lhs = nc.dram_tensor("m2lhs", [D, D], F32, kind="ExternalInput").ap()
    c2 = nc.dram_tensor("c2", [D, 1], F32, kind="ExternalInput").ap()
    i128 = nc.dram_tensor("i128", [D, D], F32, kind="ExternalInput").ap()

    cell_out = nc.dram_tensor("cell_out", [per_core_pad, D], F32,
                              kind="ExternalOutput").ap()
    tissue_out = nc.dram_tensor("tissue_out", [G_pc * LANES, D], F32,
                                kind="ExternalOutput").ap()

    with tile.TileContext(nc) as tc:
        with (
            tc.tile_pool(name="consts", bufs=1) as cpool,
            tc.tile_pool(name="gparams", bufs=2) as gpool,
            tc.tile_pool(name="stream", bufs=3) as spool,
            tc.tile_pool(name="fin", bufs=2) as fpool,
            tc.tile_pool(name="psum_mm", bufs=5, space="PSUM") as pmm,
            tc.tile_pool(name="psum_acc", bufs=1, space="PSUM") as pacc,
            tc.tile_pool(name="gath", bufs=2) as g4pool,
            tc.tile_pool(name="dram", bufs=1, space="DRAM") as dpool,
        ):
            def load_const(name, ap, shape, dtype=F32):
                t = cpool.tile(shape, dtype, name=name)
                nc.sync.dma_start(out=t[:], in_=ap[:])
                return t

            wkt_sb = load_const("wkt_sb", wkt, [D, D])
            wvt_sb = load_const("wvt_sb", wvt, [D, D])
            sp_sb = load_const("sp_sb", sprime, [D, D])
            b8_sb = load_const("b8_sb", b8, [D, H])
            bt_sb = load_const("bt_sb", bt, [H, D])
            wot_sb = load_const("wot_sb", wot, [D, D])
            m2_sb = load_const("m2_sb", m2lhs, [D, D])
            c2_sb = load_const("c2_sb", c2, [D, 1])
            id_sb = load_const("id_sb", i128, [D, D])
            idx_sb = load_const("idx_sb", idx, [128, idx_cols], I16)

            tout_loc = dpool.tile([G_pc * LANES, D], F32, name="tout_loc")
            tout_full = dpool.tile([G_total * LANES, D], F32, name="tout_full",
                                   addr_space="Shared")

            for g in range(G_pc):
                qt4_sb = gpool.tile([D, BW], F32, name="qt4_sb")
                nc.sync.dma_start(out=qt4_sb[:], in_=qt4[g])
                cm_sb = gpool.tile([LANES, LANES], F32, name="cm_sb")
                nc.sync.dma_start(out=cm_sb[:], in_=cm[g])
                nf_sb = gpool.tile([H, LANES], F32, name="nf_sb")
                nc.sync.dma_start(out=nf_sb[:], in_=nfill[g])
                m01_sb = gpool.tile([LANES, 1], F32, name="m01_sb")
                nc.sync.dma_start(out=m01_sb[:], in_=m01[g])
                h_sb = gpool.tile([LANES, LANES], F32, name="h_sb")
                nc.sync.dma_start(out=h_sb[:], in_=hmat[g])

                accV = pacc.tile([D, BW], F32, name="accV", tag="accv")
                accD = pacc.tile([H, BW], F32, name="accD", tag="accd")

                for b in range(NB):
                    xt = spool.tile([D, B, LANES], F32, name="xt")
                    nc.sync.dma_start(out=xt[:], in_=grid[g, :, b * B:(b + 1) * B, :])
                    kt = pmm.tile([D, BW], F32, name="kt", tag="mm")
                    nc.tensor.matmul(out=kt[:], lhsT=wkt_sb[:], rhs=xt[:],
                                     start=True, stop=True)
                    vt = pmm.tile([D, BW], F32, name="vt", tag="mm")
                    nc.tensor.matmul(out=vt[:], lhsT=wvt_sb[:], rhs=xt[:],
                                     start=True, stop=True)
                    prod = spool.tile([D, BW], F32, name="prod")
                    nc.vector.tensor_tensor(out=prod[:], in0=kt[:], in1=qt4_sb[:],
                                            op=mybir.AluOpType.mult)
                    sc = pmm.tile([D, BW], F32, name="sc", tag="mm")
                    nc.tensor.matmul(out=sc[:], lhsT=sp_sb[:], rhs=prod[:],
                                     start=True, stop=True)
                    et = spool.tile([D, BW], F32, name="et")
                    nc.scalar.activation(out=et[:], in_=sc[:],
                                         func=mybir.ActivationFunctionType.Exp)
                    wv = spool.tile([D, BW], F32, name="wv")
                    nc.vector.tensor_tensor(out=wv[:], in0=vt[:], in1=et[:],
                                            op=mybir.AluOpType.mult)
                    first = (b == 0)
                    last = (b == NB - 1)
                    nc.tensor.matmul(out=accV[:], lhsT=id_sb[:], rhs=wv[:],
                                     start=first, stop=last)
                    nc.tensor.matmul(out=accD[:], lhsT=b8_sb[:], rhs=et[:],
                                     start=first, stop=last)

                # ---- finalize group: fold the B sub-columns, then combine ----
                # (TensorTensor may read at most one PSUM operand, so fold
                #  via copy + running adds with the PSUM slice on in1.)
                accV_sb = fpool.tile([D, LANES], F32, name="accV_sb", tag="fsb")
                nc.vector.tensor_copy(out=accV_sb[:], in_=accV[:, 0:LANES])
                for bb in range(1, B):
                    nc.vector.tensor_tensor(out=accV_sb[:], in0=accV_sb[:],
                                            in1=accV[:, bass.ts(bb, LANES)],
                                            op=mybir.AluOpType.add)
                accD_sb = fpool.tile([H, LANES], F32, name="accD_sb", tag="fsb")
                nc.vector.tensor_copy(out=accD_sb[:], in_=accD[:, 0:LANES])
                for bb in range(1, B):
                    nc.vector.tensor_tensor(out=accD_sb[:], in0=accD_sb[:],
                                            in1=accD[:, bass.ts(bb, LANES)],
                                            op=mybir.AluOpType.add)

                accVT = pmm.tile([D, LANES], F32, name="accVT", tag="mm")
                nc.tensor.transpose(out=accVT[:], in_=accV_sb[:], identity=id_sb[:])
                accVT_sb = fpool.tile([D, LANES], F32, name="accVT_sb", tag="fsb")
                nc.vector.tensor_copy(out=accVT_sb[:], in_=accVT[:])

                accDT = pmm.tile([LANES, H], F32, name="accDT", tag="mm")
                nc.tensor.transpose(out=accDT[:], in_=accD_sb[:],
                                    identity=id_sb[:H, :H])
                accDT_sb = fpool.tile([LANES, H], F32, name="accDT_sb", tag="fsb")
                nc.vector.tensor_copy(out=accDT_sb[:], in_=accDT[:])

                combV = pmm.tile([D, LANES], F32, name="combV", tag="mm")
                nc.tensor.matmul(out=combV[:], lhsT=accVT_sb[:], rhs=cm_sb[:],
                                 start=True, stop=True)
                combD = pmm.tile([H, LANES], F32, name="combD", tag="mm")
                nc.tensor.matmul(out=combD[:], lhsT=accDT_sb[:], rhs=cm_sb[:],
                                 start=True, stop=True)

                den = fpool.tile([H, LANES], F32, name="den", tag="fsb")
                nc.vector.tensor_tensor(out=den[:], in0=combD[:], in1=nf_sb[:],
                                        op=mybir.AluOpType.subtract)
                den2 = fpool.tile([H, LANES], F32, name="den2", tag="fsb")
                nc.vector.tensor_scalar_max(out=den2[:], in0=den[:], scalar1=1e-30)
                recip = fpool.tile([H, LANES], F32, name="recip", tag="fsb")
                nc.vector.reciprocal(out=recip[:], in_=den2[:])

                reprec = pmm.tile([D, LANES], F32, name="reprec", tag="mm")
                nc.tensor.matmul(out=reprec[:], lhsT=bt_sb[:], rhs=recip[:],
                                 start=True, stop=True)
                reprec_sb = fpool.tile([D, LANES], F32, name="reprec_sb", tag="fsb")
                nc.vector.tensor_copy(out=reprec_sb[:], in_=reprec[:])

                attnT = fpool.tile([D, LANES], F32, name="attnT", tag="fsb")
                nc.vector.tensor_tensor(out=attnT[:], in0=combV[:], in1=reprec_sb[:],
                                        op=mybir.AluOpType.mult)

                bu0 = pmm.tile([LANES, D], F32, name="bu0", tag="mm")
                nc.tensor.matmul(out=bu0[:], lhsT=attnT[:], rhs=wot_sb[:],
                                 start=True, stop=True)
                tn_tmp = fpool.tile([LANES, D], F32, name="tn_tmp", tag="fsb")
                nc.vector.tensor_scalar_mul(out=tn_tmp[:], in0=bu0[:],
                                            scalar1=m01_sb[:])
                tn_sb = fpool.tile([LANES, D], F32, name="tn_sb", tag="fsb")
                nc.vector.tensor_tensor(out=tn_sb[:], in0=tn_tmp[:], in1=h_sb[:],
                                        op=mybir.AluOpType.add)
                nc.sync.dma_start(out=tissue_out[g * LANES:(g + 1) * LANES, :],
                                  in_=tn_sb[:])

                tnT = pmm.tile([D, LANES], F32, name="tnT", tag="mm")
                nc.tensor.transpose(out=tnT[:], in_=tn_sb[:], identity=id_sb[:])
                tnT_sb = fpool.tile([D, LANES], F32, name="tnT_sb", tag="fsb")
                nc.vector.tensor_copy(out=tnT_sb[:], in_=tnT[:])

                toutT = pmm.tile([D, LANES], F32, name="toutT", tag="mm")
                nc.tensor.matmul(out=toutT[:], lhsT=m2_sb[:], rhs=tnT_sb[:],
                                 start=True, stop=True)
                toutT_sb = fpool.tile([D, LANES], F32, name="toutT_sb", tag="fsb")
                nc.vector.tensor_scalar_add(out=toutT_sb[:], in0=toutT[:],
                                            scalar1=c2_sb[:])

                ton = pmm.tile([LANES, D], F32, name="ton", tag="mm")
                nc.tensor.transpose(out=ton[:], in_=toutT_sb[:], identity=id_sb[:])
                ton_sb = fpool.tile([LANES, D], F32, name="ton_sb", tag="fsb")
                nc.vector.tensor_copy(out=ton_sb[:], in_=ton[:])
                nc.sync.dma_start(out=tout_loc[g * LANES:(g + 1) * LANES, :],
                                  in_=ton_sb[:])

            # ---- share per-cluster table ----
            if single or "nocoll" in dbg:
                # timing-analysis variant: plain copy instead of AllGather
                nc.sync.dma_start(out=tout_full[:G_pc * LANES, :],
                                  in_=tout_loc[:])
            else:
                nc.gpsimd.collective_compute(
                    "AllGather", mybir.AluOpType.bypass,
                    replica_groups=[list(range(NCORES))],
                    ins=[tout_loc[:]], outs=[tout_full[:]],
                )

            # ---- pass 4: cell_out[n] = t_out[label[n]] ----
            col = 0
            for (a, nrows) in calls:
                gc = nrows // 128
                gt = g4pool.tile([128, gc, D], F32, name="gt", tag="gt")
                if "nogather" in dbg:
                    nc.gpsimd.memset(gt[:], 0.0)
                else:
                    nc.gpsimd.dma_gather(
                        out_ap=gt[:],
                        in_ap=tout_full[:],
                        idxs_ap=idx_sb[:, col: col + nrows // 16],
                        num_idxs=nrows,
                        num_idxs_reg=nrows,
                        elem_size=D,
                    )
                nc.sync.dma_start(
                    out=cell_out[a:a + nrows, :].rearrange(
                        "(p g) e -> p g e", p=128),
                    in_=gt[:],
                )
                col += nrows // 16

    _log(f"tile emission+schedule: {time.time()-t0:.1f}s")
    t0 = time.time()
    nc.compile()
    _log(f"bacc compile: {time.time()-t0:.1f}s")
    _CACHE[key] = nc
    return nc


# --------------------------------------------------------------------------
# entry point
# --------------------------------------------------------------------------

def kernel(cell_features, tissue_features, cluster_labels, tissue_batch,
           num_heads,
           bu_in_w, bu_in_b, bu_out_w, bu_out_b,
           td_in_w, td_in_b, td_out_w, td_out_b,
           _use_sim=False, _trace=False):
    assert int(num_heads) == H
    t0 = time.time()
    host, maps = _host_prepare(
        np.asarray(cell_features), np.asarray(tissue_features),
        np.asarray(cluster_labels),
        np.asarray(bu_in_w), np.asarray(bu_in_b),
        np.asarray(bu_out_w), np.asarray(bu_out_b),
        np.asarray(td_in_w), np.asarray(td_in_b),
        np.asarray(td_out_w), np.asarray(td_out_b))
    _log(f"host_prepare: {time.time()-t0:.1f}s  G_total={host['G_total']}")

    nc = _build(host["G_pc"], host["G_total"], host["per_core_pad"],
                host["idx_cols"], host["calls"])

    globals()["LAST_NC"] = nc
    globals()["LAST_MAPS"] = maps
    globals()["LAST_HOST"] = host

    if _use_sim:
        from concourse.bass_interp import MultiCoreSim
        sim = MultiCoreSim(nc, num_cores=NCORES, trace=False,
                           require_finite=False, require_nnan=False)
        for r, core in sim.cores.items():
            for k, v in maps[r].items():
                core.tensor(k)[:] = v
        sim.simulate(check_with_hw=False)
        results = [{"cell_out": np.array(sim.cores[r].tensor("cell_out")),
                    "tissue_out": np.array(sim.cores[r].tensor("tissue_out"))}
                   for r in range(NCORES)]
    else:
        t0 = time.time()
        br = run_bass_kernel_spmd(nc, maps, core_ids=list(range(NCORES)),
                                  trace=_trace)
        _log(f"hw run (incl jit/neff+transfers): {time.time()-t0:.1f}s")
        globals()["LAST_RESULTS"] = br
        results = br.results

    N, C = host["N"], host["C"]
    per_core, G_pc = host["per_core"], host["G_pc"]
    cell_new = np.concatenate(
        [results[r]["cell_out"][:min(per_core, N - r * per_core)]
         for r in range(NCORES)], axis=0)

    tissue_new = np.array(host["tis"], np.float32, copy=True)
    sc = host["slot_cluster"]
    for r in range(NCORES):
        tiss = results[r]["tissue_out"]
        for gl in range(G_pc):
            gg = r * G_pc + gl
            for o in range(LANES):
                c = sc[gg, o]
                if c >= 0:
                    tissue_new[c] = tiss[gl * LANES + o]

    valid = host["valid"]
    if not valid.all():
        cell_new[~valid] = host["X"][~valid]

    return (np.asarray(cell_new, np.float32),
            np.asarray(tissue_new, np.float32))


# --------------------------------------------------------------------------
# timing: chain n NEFF executions inside one jit call; slope = per-exec time
# --------------------------------------------------------------------------

def bench_baseline(n_repeat=6):
    """Time a trivial 8-core bass kernel through the identical run path to
    estimate the fixed axon-RPC/dispatch overhead per execution."""
    nc2 = bacc.Bacc("TRN2", target_bir_lowering=False, debug=False,
                    num_devices=NCORES)
    a = nc2.dram_tensor("a", [128, 128], F32, kind="ExternalInput").ap()
    b = nc2.dram_tensor("b", [128, 128], F32, kind="ExternalOutput").ap()
    with tile.TileContext(nc2) as tc:
        with tc.tile_pool(name="p", bufs=1) as p:
            t = p.tile([128, 128], F32, name="t")
            nc2.sync.dma_start(out=t[:], in_=a[:])
            nc2.sync.dma_start(out=b[:], in_=t[:])
    nc2.compile()
    maps2 = [{"a": np.zeros((128, 128), np.float32)} for _ in range(NCORES)]
    return bench(n_chain=1, n_repeat=n_repeat, nc=nc2, maps=maps2)[1][1]


def bench(n_chain=9, n_repeat=4, nc=None, maps=None):
    import jax
    from jax.sharding import Mesh, PartitionSpec, NamedSharding
    from concourse import bass2jax

    if nc is None:
        nc, maps = LAST_NC, LAST_MAPS
    bass2jax.install_neuronx_cc_hook()
    partition_name = nc.partition_id_tensor.name if nc.partition_id_tensor else None

    in_names, out_names, out_avals, zero_outs = [], [], [], []
    for alloc in nc.m.functions[0].allocations:
        if not isinstance(alloc, mybir.MemoryLocationSet):
            continue
        name = alloc.memorylocations[0].name
        if alloc.kind == "ExternalInput":
            if name != partition_name:
                in_names.append(name)
        elif alloc.kind == "ExternalOutput":
            out_names.append(name)
            shape = tuple(alloc.tensor_shape)
            dtype = mybir.dt.np(alloc.dtype)
            out_avals.append(jax.core.ShapedArray(shape, dtype))
            zero_outs.append(np.zeros(shape, dtype))
    n_params, n_outs = len(in_names), len(out_avals)
    all_in_names = list(in_names) + out_names + (
        [partition_name] if partition_name else [])

    devices = jax.devices()[:NCORES]
    mesh = Mesh(np.asarray(devices), ("core",))
    shard = NamedSharding(mesh, PartitionSpec("core"))

    concat_in = [np.concatenate([maps[c][n] for c in range(NCORES)], axis=0)
                 for n in in_names]
    dev_in = [jax.device_put(a, shard) for a in concat_in]
    dev_zero = [jax.device_put(
        np.zeros((NCORES * z.shape[0], *z.shape[1:]), z.dtype), shard)
        for z in zero_outs]

    from jax.experimental.shard_map import shard_map

    def make(nch):
        def _chain(*args):
            ins = list(args[:n_params])
            outs = list(args[n_params:])
            for _ in range(nch):
                operands = ins + outs
                if partition_name is not None:
                    operands.append(bass2jax.partition_id_tensor())
                outs = list(bass2jax._bass_exec_p.bind(
                    *operands,
                    out_avals=tuple(out_avals),
                    in_names=tuple(all_in_names),
                    out_names=tuple(out_names),
                    lowering_input_output_aliases=(),
                    sim_require_finite=True,
                    sim_require_nnan=True,
                    nc=nc,
                ))
            return tuple(outs)

        in_specs = (PartitionSpec("core"),) * (n_params + n_outs)
        out_specs = (PartitionSpec("core"),) * n_outs
        return jax.jit(shard_map(_chain, mesh=mesh, in_specs=in_specs,
                                 out_specs=out_specs, check_rep=False))

    results = {}
    for nch in {1, n_chain}:
        fn = make(nch)
        out = fn(*dev_in, *dev_zero)   # compile + warm
        jax.block_until_ready(out)
        times = []
        for _ in range(n_repeat):
            t0 = time.perf_counter()
            out = fn(*dev_in, *dev_zero)
            jax.block_until_ready(out)
            times.append(time.perf_counter() - t0)
        results[nch] = min(times)
        _log(f"bench chain={nch}: {[f'{t*1e3:.2f}ms' for t in times]}")
    if n_chain > 1:
        per_exec = (results[n_chain] - results[1]) / (n_chain - 1)
    else:
        per_exec = results[1]
    return per_exec, results
